# revision 7
# baseline (speedup 1.0000x reference)
"""Trainium2 Bass kernel for nn_MHA_2688649527670.

Reference computes, per batch b and head h:
    Q = x Wq_h^T, K = x Wk_h^T, V = x Wv_h^T          ([S, D] each)
    Z = softmax_over_d( (Q K^T / sqrt(D)) V )

There is NO softmax between Q K^T and V, so the chain is associative:
    (Q K^T) V = x * (Wq_h^T Wk_h G Wv_h^T) / sqrt(D),   G = x^T x   ([D, D])

This collapses the O(S^2 D) attention into a [D,D] weight-chain plus one
[S,D]x[D,D] matmul per head, followed by softmax over the model dim (free
axis, per-head bias mandatory: per-head logit scales differ by >1000x so a
shared row max underflows weak heads).

Sharding: data parallel over batch (4) x tensor parallel over head-groups
(2 groups of 4 heads) = 8 cores, fully independent.

Perf structure (vs the 59.9us fp32 baseline):
  - finals + G + UT in float32r (1 cycle/row at N>=256 vs 4 for fp32)
  - G accumulated as [G|G] (N=256) to hit the f32r fast path
  - PE p-state warmup transposes (keep the streak alive -> 2.4 GHz)
  - per-head Exp on scalar engine; single 4-head sum reduce on vector;
    paired reciprocals; normalize-multiply on gpsimd with bf16 output
  - s-major contiguous bf16 output DMA (host transposes + upcasts)
"""

import numpy as np

import concourse.bass as bass
import concourse.bacc as bacc
import concourse.mybir as mybir
import concourse.tile as tile
from concourse.bass_utils import run_bass_kernel_spmd
from concourse.masks import make_identity

B, S, D, H = 4, 2048, 128, 8
P = 128
HPC = H // 2          # heads per core
NCH = S // P          # 16 s-chunks of 128 rows
N_CORES = 8
SCALE = 1.0 / float(np.sqrt(D))
F32 = mybir.dt.float32
F32R = mybir.dt.float32r
BF16 = mybir.dt.bfloat16

# accuracy/perf toggles (fp32r = reduced-precision single-pass fp32 matmul;
# the producing instruction must WRITE f32r so values are rounded -- psum->sbuf
# copies do the conversion for free)
FINALS_F32R = False
UT_F32R = False
N_WARM = 6

_PROG = None


def _build_program():
    nc = bacc.Bacc("TRN2", target_bir_lowering=False, debug=False,
                   num_devices=N_CORES)

    x_d = nc.dram_tensor("x", [S, D], F32, kind="ExternalInput")
    wq_d = nc.dram_tensor("wq", [HPC * D, D], F32, kind="ExternalInput")
    wk_d = nc.dram_tensor("wk", [HPC * D, D], F32, kind="ExternalInput")
    wv_d = nc.dram_tensor("wv", [HPC * D, D], F32, kind="ExternalInput")
    # s-major output: [chunk, s-in-chunk, head, d], bf16; host reorders
    out_d = nc.dram_tensor("out", [NCH, P, HPC, D], BF16, kind="ExternalOutput")

    XT_DT = F32R if FINALS_F32R else F32

    with tile.TileContext(nc) as tc:
        with (
            tc.tile_pool(name="const", bufs=1) as const,
            tc.tile_pool(name="work", bufs=4) as work,
            tc.tile_pool(name="small", bufs=2) as small,
            tc.tile_pool(name="ps_y", bufs=3, space="PSUM") as ps_y,
            tc.tile_pool(name="ps_g", bufs=1, space="PSUM") as ps_g,
            tc.tile_pool(name="ps_c", bufs=2, space="PSUM") as ps_c,
            tc.tile_pool(name="ps_t", bufs=2, space="PSUM") as ps_t,
        ):
            ident = const.tile([P, P], F32, tag="ident")
            make_identity(nc, ident)

            # ---- input DMAs ----
            w_sb = {}
            for nm, wd in (("wq", wq_d), ("wk", wk_d), ("wv", wv_d)):
                t = const.tile([P, HPC, D], F32, tag=f"{nm}_sb", name=f"{nm}_sb")
                nc.gpsimd.dma_start(t, wd.ap().rearrange("(h p) c -> p h c", p=P))
                w_sb[nm] = t

            x_sb = const.tile([P, NCH, D], F32, tag="x_sb")
            x_view = x_d.ap().rearrange("(n p) c -> p n c", p=P)
            for q in range(8):
                eng = nc.sync if q % 2 == 0 else nc.scalar
                eng.dma_start(x_sb[:, 2 * q:2 * q + 2, :],
                              x_view[:, 2 * q:2 * q + 2, :])

            # ---- PE warmup (p-state ramp: keep the tensor engine busy) ----
            for w in range(N_WARM):
                warm = ps_t.tile([P, P], F32, tag="tp")
                nc.tensor.transpose(warm, ident, ident)

            # ---- weight-only chain prefix: P0T, WvT (overlap x DMA) ----
            # P0T_h = Wk_h^T Wq_h ; WvT_h = Wv_h^T
            p0t_ps = ps_c.tile([P, HPC * D], F32, tag="c_ps")
            for h in range(HPC):
                nc.tensor.matmul(p0t_ps[:, h * D:(h + 1) * D],
                                 lhsT=w_sb["wk"][:, h, :], rhs=w_sb["wq"][:, h, :])
            p0t_sb = const.tile([P, HPC * D], F32R if UT_F32R else F32,
                                tag="p0t_sb")
            nc.vector.tensor_copy(p0t_sb, p0t_ps)

            wvt_ps = ps_c.tile([P, HPC * D], F32, tag="c_ps")
            for h in range(HPC):
                nc.tensor.transpose(wvt_ps[:, h * D:(h + 1) * D],
                                    w_sb["wv"][:, h, :], ident)
            wvt_sb = const.tile([P, HPC * D], F32, tag="wvt_sb")
            nc.vector.tensor_copy(wvt_sb, wvt_ps)

            # ---- G = x^T x (accumulated over 16 s-chunks, DMA-paced),
            #      interleaved with xT transposes (copies write f32r) ----
            g_ps = ps_g.tile([P, P], F32, tag="g_ps")
            xT_sb = const.tile([P, NCH, D], XT_DT, tag="xT_sb")
            for i in range(NCH):
                xc = x_sb[:, i, :]
                nc.tensor.matmul(g_ps, lhsT=xc, rhs=xc,
                                 start=(i == 0), stop=(i == NCH - 1))
                tp = ps_t.tile([P, P], F32, tag="tp")
                nc.tensor.transpose(tp, xc, ident)
                if i % 2 == 0:
                    nc.vector.tensor_copy(xT_sb[:, i, :], tp)
                else:
                    nc.scalar.copy(xT_sb[:, i, :], tp)

            g_sb = const.tile([P, P], F32R if UT_F32R else F32, tag="g_sb")
            nc.vector.tensor_copy(g_sb, g_ps)

            # ---- UT = G @ P0T (G symmetric), all 4 heads in one matmul ----
            ut_ps = ps_c.tile([P, HPC * D], F32, tag="c_ps")
            nc.tensor.matmul(ut_ps, lhsT=g_sb, rhs=p0t_sb)
            ut_sb = const.tile([P, HPC * D], F32, tag="ut_sb")
            nc.vector.tensor_copy(ut_sb, ut_ps)

            # ---- M_h = UT_h^T WvT_h, scaled ----
            m_ps = ps_c.tile([P, HPC * D], F32, tag="c_ps")
            for h in range(HPC):
                sl = slice(h * D, (h + 1) * D)
                nc.tensor.matmul(m_ps[:, sl], lhsT=ut_sb[:, sl],
                                 rhs=wvt_sb[:, sl])
            m_all = const.tile([P, HPC * D], XT_DT, tag="m_all")
            nc.vector.tensor_scalar_mul(m_all, m_ps, SCALE)

            # ---- finals + softmax epilogue, processed in pairs ----
            for k in range(NCH // 2):
                pair = (2 * k, 2 * k + 1)
                t_tiles, y_tiles, sums = {}, {}, None
                sums = small.tile([P, 2, HPC], F32, tag="sums")
                for j, i in enumerate(pair):
                    y_ps = ps_y.tile([P, HPC * D], F32, tag="y_ps")
                    nc.tensor.matmul(y_ps, lhsT=xT_sb[:, i, :], rhs=m_all[:])
                    y_tiles[j] = y_ps

                    negmax = small.tile([P, HPC], F32, tag="negmax")
                    nc.vector.reduce_max(
                        out=negmax,
                        in_=y_ps[:].rearrange("p (h d) -> p h d", h=HPC),
                        axis=mybir.AxisListType.X, negate=True)

                    t_sb = work.tile([P, HPC, D], F32, tag="t_sb")
                    for h in range(HPC):
                        nc.scalar.activation(
                            t_sb[:, h, :], y_ps[:, h * D:(h + 1) * D],
                            mybir.ActivationFunctionType.Exp,
                            bias=negmax[:, h:h + 1], scale=1.0)
                    t_tiles[j] = t_sb

                    nc.vector.reduce_sum(out=sums[:, j], in_=t_sb,
                                         axis=mybir.AxisListType.X)

                seps = small.tile([P, 2, HPC], F32, tag="seps")
                nc.gpsimd.tensor_scalar_add(seps, sums, 1e-30)
                rsum = small.tile([P, 2, HPC], F32, tag="rsum")
                nc.vector.reciprocal(rsum, seps)

                for j, i in enumerate(pair):
                    o_sb = work.tile([P, HPC, D], BF16, tag="o_sb")
                    nc.gpsimd.tensor_tensor(
                        o_sb, t_tiles[j],
                        rsum[:, j][:, :, None].to_broadcast((P, HPC, D)),
                        mybir.AluOpType.mult)
                    eng = nc.sync if i % 2 == 0 else nc.scalar
                    eng.dma_start(out_d.ap()[i], o_sb)

    nc.compile()
    return nc


def _get_program():
    global _PROG
    if _PROG is None:
        _PROG = _build_program()
    return _PROG


def _make_in_maps(x, W_q, W_k, W_v):
    in_maps = []
    for core in range(N_CORES):
        b, hg = core // 2, core % 2
        sl = slice(hg * HPC * D, (hg + 1) * HPC * D)
        in_maps.append({
            "x": np.ascontiguousarray(x[b]),
            "wq": np.ascontiguousarray(W_q[sl]),
            "wk": np.ascontiguousarray(W_k[sl]),
            "wv": np.ascontiguousarray(W_v[sl]),
        })
    return in_maps


def run(x, W_q, W_k, W_v, trace=False, **spmd_kwargs):
    """Run on 8 NeuronCores; returns (Z, BassKernelResults)."""
    nc = _get_program()
    in_maps = _make_in_maps(np.asarray(x, np.float32), np.asarray(W_q, np.float32),
                            np.asarray(W_k, np.float32), np.asarray(W_v, np.float32))
    res = run_bass_kernel_spmd(nc, in_maps, core_ids=list(range(N_CORES)),
                               trace=trace, **spmd_kwargs)
    Z = np.empty((B, H, S, D), np.float32)
    for core in range(N_CORES):
        b, hg = core // 2, core % 2
        o = np.asarray(res.results[core]["out"]).astype(np.float32)
        # [NCH, P, HPC, D] -> [HPC, NCH*P, D]
        Z[b, hg * HPC:(hg + 1) * HPC] = o.transpose(2, 0, 1, 3).reshape(HPC, S, D)
    return Z, res


def kernel(x, W_q, W_k, W_v):
    Z, _ = run(x, W_q, W_k, W_v, trace=False)
    return Z


# revision 8
# speedup vs baseline: 1.1990x; 1.1990x over previous
"""Trainium2 Bass kernel for nn_MHA_2688649527670.

Reference computes, per batch b and head h:
    Q = x Wq_h^T, K = x Wk_h^T, V = x Wv_h^T          ([S, D] each)
    Z = softmax_over_d( (Q K^T / sqrt(D)) V )

No softmax between Q K^T and V, so the chain is associative:
    (Q K^T) V = x (Wq_h^T Wk_h G Wv_h^T) / sqrt(D),   G = x^T x   ([D, D])

which collapses the O(S^2 D) attention into a [D,D] weight chain plus one
[S,D]x[D,D*H] matmul, then softmax over d (free axis). Per-head softmax bias
is mandatory: per-head logit scales differ by >1000x, so a shared row max
underflows weak heads.

Sharding: batch (4) x head-groups (2x4 heads) = 8 independent cores.

Perf notes:
  - finals/UT in float32r: ~1 cycle/row at N=512 vs 4 for fp32, and HW
    measures ~1.5e-4 matmul error (~16x better than bf16; bf16 finals fail
    the 2e-2 gate at 3.1e-2, f32r lands ~2e-3). f32r operands must be
    WRITTEN as f32r by their producer (BIR rule); psum->sbuf copies do it.
  - PE p-state warmup matmuls on a memset tile from t~0 (streak -> 2.4GHz).
  - PE order: G (DMA-paced) with xT transposes interleaved, then P0T/WvT
    (weights arrive late on the gpsimd queue - off critical path), UT, M.
  - epilogue per chunk: V reduce_max -> 4x scalar Exp (per-head bias) ->
    V reduce_sum -> paired V reciprocal -> gpsimd normalize-mult (bf16) ->
    s-major contiguous bf16 DMA out (host reorders/upcasts).
"""

import numpy as np

import concourse.bass as bass
import concourse.bacc as bacc
import concourse.mybir as mybir
import concourse.tile as tile
from concourse.bass_utils import run_bass_kernel_spmd
from concourse.masks import make_identity

B, S, D, H = 4, 2048, 128, 8
P = 128
HPC = H // 2          # heads per core
NCH = S // P          # 16 s-chunks of 128 rows
N_CORES = 8
SCALE = 1.0 / float(np.sqrt(D))
F32 = mybir.dt.float32
F32R = mybir.dt.float32r
BF16 = mybir.dt.bfloat16

FINALS_F32R = True
UT_F32R = True
N_WARM = 10

_PROG = None


def _build_program():
    nc = bacc.Bacc("TRN2", target_bir_lowering=False, debug=False,
                   num_devices=N_CORES)

    x_d = nc.dram_tensor("x", [S, D], F32, kind="ExternalInput")
    wq_d = nc.dram_tensor("wq", [HPC * D, D], F32, kind="ExternalInput")
    wk_d = nc.dram_tensor("wk", [HPC * D, D], F32, kind="ExternalInput")
    wv_d = nc.dram_tensor("wv", [HPC * D, D], F32, kind="ExternalInput")
    # s-major output: [chunk, s-in-chunk, head, d], bf16; host reorders
    out_d = nc.dram_tensor("out", [NCH, P, HPC, D], BF16, kind="ExternalOutput")

    XT_DT = F32R if FINALS_F32R else F32

    with tile.TileContext(nc) as tc:
        with (
            tc.tile_pool(name="const", bufs=1) as const,
            tc.tile_pool(name="work", bufs=4) as work,
            tc.tile_pool(name="small", bufs=2) as small,
            tc.tile_pool(name="ps_y", bufs=3, space="PSUM") as ps_y,
            tc.tile_pool(name="ps_g", bufs=1, space="PSUM") as ps_g,
            tc.tile_pool(name="ps_t", bufs=2, space="PSUM") as ps_t,
            tc.tile_pool(name="ps_w", bufs=2, space="PSUM") as ps_w,
        ):
            ident = const.tile([P, P], F32, tag="ident")
            make_identity(nc, ident)

            # ---- input DMAs: x on the two HW queues, weights on gpsimd ----
            x_sb = const.tile([P, NCH, D], F32, tag="x_sb")
            x_view = x_d.ap().rearrange("(n p) c -> p n c", p=P)
            for q in range(8):
                eng = nc.sync if q % 2 == 0 else nc.scalar
                eng.dma_start(x_sb[:, 2 * q:2 * q + 2, :],
                              x_view[:, 2 * q:2 * q + 2, :])
            w_sb = {}
            for nm, wd in (("wq", wq_d), ("wk", wk_d), ("wv", wv_d)):
                t = const.tile([P, HPC, D], F32, tag=f"{nm}_sb", name=f"{nm}_sb")
                nc.gpsimd.dma_start(t, wd.ap().rearrange("(h p) c -> p h c", p=P))
                w_sb[nm] = t

            # ---- PE warmup from t~0: matmuls on a vector-memset tile ----
            wsrc = const.tile([P, P], F32, tag="wsrc")
            nc.vector.memset(wsrc, 0.0)
            for w in range(N_WARM):
                warm = ps_w.tile([P, P], F32, tag="warm")
                nc.tensor.matmul(warm, lhsT=wsrc, rhs=wsrc)

            # ---- G = x^T x (DMA-paced) with xT transposes interleaved ----
            g_ps = ps_g.tile([P, P], F32, tag="g_ps")
            xT_sb = const.tile([P, NCH, D], XT_DT, tag="xT_sb")

            def emit_xt(i):
                tp = ps_t.tile([P, P], F32, tag="tp")
                nc.tensor.transpose(tp, x_sb[:, i, :], ident)
                if i % 2 == 0:
                    nc.vector.tensor_copy(xT_sb[:, i, :], tp)
                else:
                    nc.scalar.copy(xT_sb[:, i, :], tp)

            for i in range(NCH):
                nc.tensor.matmul(g_ps, lhsT=x_sb[:, i, :], rhs=x_sb[:, i, :],
                                 start=(i == 0), stop=(i == NCH - 1))
                if i >= 4:
                    emit_xt(i - 4)
            for i in range(NCH - 4, NCH):
                emit_xt(i)

            # ---- weight-only chain parts (weights arrive by now) ----
            p0t_ps = ps_y.tile([P, HPC * D], F32, tag="c_ps")
            for h in range(HPC):
                nc.tensor.matmul(p0t_ps[:, h * D:(h + 1) * D],
                                 lhsT=w_sb["wk"][:, h, :], rhs=w_sb["wq"][:, h, :])
            p0t_sb = const.tile([P, HPC * D], F32R if UT_F32R else F32,
                                tag="p0t_sb")
            nc.vector.tensor_copy(p0t_sb, p0t_ps)

            wvt_ps = ps_y.tile([P, HPC * D], F32, tag="c_ps")
            for h in range(HPC):
                nc.tensor.transpose(wvt_ps[:, h * D:(h + 1) * D],
                                    w_sb["wv"][:, h, :], ident)
            wvt_sb = const.tile([P, HPC * D], F32, tag="wvt_sb")
            nc.vector.tensor_copy(wvt_sb, wvt_ps)

            g_sb = const.tile([P, P], F32R if UT_F32R else F32, tag="g_sb")
            nc.vector.tensor_copy(g_sb, g_ps)

            # ---- UT = G @ P0T (G symmetric), one N=512 matmul ----
            ut_ps = ps_y.tile([P, HPC * D], F32, tag="c_ps")
            nc.tensor.matmul(ut_ps, lhsT=g_sb, rhs=p0t_sb)
            ut_sb = const.tile([P, HPC * D], F32, tag="ut_sb")
            nc.vector.tensor_copy(ut_sb, ut_ps)

            # ---- M_h = UT_h^T WvT_h, scaled on the scalar engine ----
            m_ps = ps_y.tile([P, HPC * D], F32, tag="c_ps")
            for h in range(HPC):
                sl = slice(h * D, (h + 1) * D)
                nc.tensor.matmul(m_ps[:, sl], lhsT=ut_sb[:, sl],
                                 rhs=wvt_sb[:, sl])
            m_all = const.tile([P, HPC * D], XT_DT, tag="m_all")
            nc.scalar.mul(m_all, m_ps, SCALE)

            # ---- finals + softmax epilogue, paired for the reciprocal ----
            for k in range(NCH // 2):
                pair = (2 * k, 2 * k + 1)
                t_tiles = {}
                sums = small.tile([P, 2, HPC], F32, tag="sums")
                for j, i in enumerate(pair):
                    y_ps = ps_y.tile([P, HPC * D], F32, tag="c_ps")
                    nc.tensor.matmul(y_ps, lhsT=xT_sb[:, i, :], rhs=m_all[:])

                    negmax = small.tile([P, HPC], F32, tag="negmax")
                    nc.vector.reduce_max(
                        out=negmax,
                        in_=y_ps[:].rearrange("p (h d) -> p h d", h=HPC),
                        axis=mybir.AxisListType.X, negate=True)

                    t_sb = work.tile([P, HPC, D], F32, tag="t_sb")
                    for h in range(HPC):
                        nc.scalar.activation(
                            t_sb[:, h, :], y_ps[:, h * D:(h + 1) * D],
                            mybir.ActivationFunctionType.Exp,
                            bias=negmax[:, h:h + 1], scale=1.0)
                    t_tiles[j] = t_sb

                    nc.vector.reduce_sum(out=sums[:, j], in_=t_sb,
                                         axis=mybir.AxisListType.X)

                rsum = small.tile([P, 2, HPC], F32, tag="rsum")
                nc.vector.reciprocal(rsum, sums)

                for j, i in enumerate(pair):
                    o_sb = work.tile([P, HPC, D], BF16, tag="o_sb")
                    nc.gpsimd.tensor_tensor(
                        o_sb, t_tiles[j],
                        rsum[:, j][:, :, None].to_broadcast((P, HPC, D)),
                        mybir.AluOpType.mult)
                    eng = nc.sync if i % 2 == 0 else nc.scalar
                    eng.dma_start(out_d.ap()[i], o_sb)

    nc.compile()
    return nc


def _get_program():
    global _PROG
    if _PROG is None:
        _PROG = _build_program()
    return _PROG


def _make_in_maps(x, W_q, W_k, W_v):
    in_maps = []
    for core in range(N_CORES):
        b, hg = core // 2, core % 2
        sl = slice(hg * HPC * D, (hg + 1) * HPC * D)
        in_maps.append({
            "x": np.ascontiguousarray(x[b]),
            "wq": np.ascontiguousarray(W_q[sl]),
            "wk": np.ascontiguousarray(W_k[sl]),
            "wv": np.ascontiguousarray(W_v[sl]),
        })
    return in_maps


def run(x, W_q, W_k, W_v, trace=False, **spmd_kwargs):
    """Run on 8 NeuronCores; returns (Z, BassKernelResults)."""
    nc = _get_program()
    in_maps = _make_in_maps(np.asarray(x, np.float32), np.asarray(W_q, np.float32),
                            np.asarray(W_k, np.float32), np.asarray(W_v, np.float32))
    res = run_bass_kernel_spmd(nc, in_maps, core_ids=list(range(N_CORES)),
                               trace=trace, **spmd_kwargs)
    Z = np.empty((B, H, S, D), np.float32)
    for core in range(N_CORES):
        b, hg = core // 2, core % 2
        o = np.asarray(res.results[core]["out"]).astype(np.float32)
        # [NCH, P, HPC, D] -> [HPC, NCH*P, D]
        Z[b, hg * HPC:(hg + 1) * HPC] = o.transpose(2, 0, 1, 3).reshape(HPC, S, D)
    return Z, res


def kernel(x, W_q, W_k, W_v):
    Z, _ = run(x, W_q, W_k, W_v, trace=False)
    return Z


# revision 10
# speedup vs baseline: 1.4890x; 1.2418x over previous
"""Trainium2 Bass kernel for nn_MHA_2688649527670.

Reference computes, per batch b and head h:
    Q = x Wq_h^T, K = x Wk_h^T, V = x Wv_h^T          ([S, D] each)
    Z = softmax_over_d( (Q K^T / sqrt(D)) V )

No softmax between Q K^T and V, so the chain is associative:
    (Q K^T) V = x (Wq_h^T Wk_h G Wv_h^T) / sqrt(D),   G = x^T x   ([D, D])

which collapses the O(S^2 D) attention into a [D,D] weight chain plus one
[S,D]x[D,D*H] matmul, then softmax over d (free axis). Per-head softmax bias
is mandatory: per-head logit scales differ by >1000x, so a shared row max
underflows weak heads.

Sharding: batch (4) x head-groups (2x4 heads) = 8 independent cores.

Perf notes:
  - finals/UT in float32r: ~1 cycle/row at N=512 vs 4 for fp32, and HW
    measures ~1.5e-4 matmul error (~16x better than bf16; bf16 finals fail
    the 2e-2 gate at 3.1e-2, f32r lands ~2e-3). f32r operands must be
    WRITTEN as f32r by their producer (BIR rule); psum->sbuf copies do it.
  - PE p-state warmup matmuls on a memset tile from t~0 (streak -> 2.4GHz).
  - PE order: G (DMA-paced) with xT transposes interleaved, then P0T/WvT
    (weights arrive late on the gpsimd queue - off critical path), UT, M.
  - epilogue per chunk: V reduce_max -> 4x scalar Exp (per-head bias) ->
    V reduce_sum -> paired V reciprocal -> gpsimd normalize-mult (bf16) ->
    s-major contiguous bf16 DMA out (host reorders/upcasts).
"""

import numpy as np

import concourse.bass as bass
import concourse.bacc as bacc
import concourse.mybir as mybir
import concourse.tile as tile
from concourse.bass_utils import run_bass_kernel_spmd
from concourse.masks import make_identity

B, S, D, H = 4, 2048, 128, 8
P = 128
HPC = H // 2          # heads per core
NCH = S // P          # 16 s-chunks of 128 rows
N_CORES = 8
SCALE = 1.0 / float(np.sqrt(D))
F32 = mybir.dt.float32
F32R = mybir.dt.float32r
BF16 = mybir.dt.bfloat16

FINALS_F32R = True
UT_F32R = True
N_WARM = 4

_PROG = None


def _build_program():
    nc = bacc.Bacc("TRN2", target_bir_lowering=False, debug=False,
                   num_devices=N_CORES)

    x_d = nc.dram_tensor("x", [S, D], F32, kind="ExternalInput")
    wq_d = nc.dram_tensor("wq", [HPC * D, D], F32, kind="ExternalInput")
    wk_d = nc.dram_tensor("wk", [HPC * D, D], F32, kind="ExternalInput")
    wv_d = nc.dram_tensor("wv", [HPC * D, D], F32, kind="ExternalInput")
    # s-major output: [chunk, s-in-chunk, head, d], bf16; host reorders
    out_d = nc.dram_tensor("out", [NCH, P, HPC, D], BF16, kind="ExternalOutput")

    XT_DT = F32R if FINALS_F32R else F32

    with tile.TileContext(nc) as tc:
        with (
            tc.tile_pool(name="const", bufs=1) as const,
            tc.tile_pool(name="work", bufs=6) as work,
            tc.tile_pool(name="small", bufs=3) as small,
            tc.tile_pool(name="ps_y", bufs=4, space="PSUM") as ps_y,
            tc.tile_pool(name="ps_g", bufs=1, space="PSUM") as ps_g,
            tc.tile_pool(name="ps_t", bufs=3, space="PSUM") as ps_t,
        ):
            ident = const.tile([P, P], F32, tag="ident")
            make_identity(nc, ident)

            # ---- input DMAs: x on the two HW queues, weights on gpsimd ----
            x_sb = const.tile([P, NCH, D], F32, tag="x_sb")
            x_view = x_d.ap().rearrange("(n p) c -> p n c", p=P)
            for q in range(8):
                eng = nc.sync if q % 2 == 0 else nc.scalar
                eng.dma_start(x_sb[:, 2 * q:2 * q + 2, :],
                              x_view[:, 2 * q:2 * q + 2, :])
            w_sb = {}
            for nm, wd in (("wq", wq_d), ("wk", wk_d), ("wv", wv_d)):
                t = const.tile([P, HPC, D], F32, tag=f"{nm}_sb", name=f"{nm}_sb")
                nc.gpsimd.dma_start(t, wd.ap().rearrange("(h p) c -> p h c", p=P))
                w_sb[nm] = t

            # ---- PE warmup from t~0: matmuls on a vector-memset tile,
            #      recycled through the g bank (nothing reads them) ----
            wsrc = const.tile([P, P], F32, tag="wsrc")
            nc.vector.memset(wsrc, 0.0)
            for w in range(N_WARM):
                warm = ps_g.tile([P, P], F32, tag="g_ps")
                nc.tensor.matmul(warm, lhsT=wsrc, rhs=wsrc)

            # ---- G = x^T x (DMA-paced) with xT transposes interleaved.
            #      Transposes land 4-per-PSUM-bank; one V cast per bank
            #      writes the f32r xT tile (16 copies -> 4). ----
            g_ps = ps_g.tile([P, P], F32, tag="g_ps")
            xT_sb = const.tile([P, NCH, D], XT_DT, tag="xT_sb")
            tp_banks = {}

            def emit_xt(i):
                b = i // 4
                if i % 4 == 0:
                    tp_banks[b] = ps_t.tile([P, 4, P], F32, tag="tp", name=f"tp{b}")
                nc.tensor.transpose(tp_banks[b][:, i % 4, :],
                                    x_sb[:, i, :], ident)
                if i % 4 == 3:
                    nc.vector.tensor_copy(
                        xT_sb[:, 4 * b:4 * b + 4, :], tp_banks[b])

            p0t_ps = ps_y.tile([P, HPC * D], F32, tag="c_ps")
            for i in range(NCH):
                nc.tensor.matmul(g_ps, lhsT=x_sb[:, i, :], rhs=x_sb[:, i, :],
                                 start=(i == 0), stop=(i == NCH - 1))
                if i >= 4:
                    emit_xt(i - 4)
                if i >= 12:  # P0T woven into the G tail (weights ready)
                    h = i - 12
                    nc.tensor.matmul(p0t_ps[:, h * D:(h + 1) * D],
                                     lhsT=w_sb["wk"][:, h, :],
                                     rhs=w_sb["wq"][:, h, :])
            for i in range(NCH - 4, NCH):
                emit_xt(i)
            p0t_sb = const.tile([P, HPC * D], F32R if UT_F32R else F32,
                                tag="p0t_sb")
            nc.vector.tensor_copy(p0t_sb, p0t_ps)

            g_sb = const.tile([P, P], F32R if UT_F32R else F32, tag="g_sb")
            nc.vector.tensor_copy(g_sb, g_ps)

            wvt_ps = ps_y.tile([P, HPC * D], F32, tag="c_ps")
            for h in range(HPC):
                nc.tensor.transpose(wvt_ps[:, h * D:(h + 1) * D],
                                    w_sb["wv"][:, h, :], ident)
            wvt_sb = const.tile([P, HPC * D], F32, tag="wvt_sb")
            nc.vector.tensor_copy(wvt_sb, wvt_ps)

            # ---- UT = G @ P0T (G symmetric), one N=512 matmul ----
            ut_ps = ps_y.tile([P, HPC * D], F32, tag="c_ps")
            nc.tensor.matmul(ut_ps, lhsT=g_sb, rhs=p0t_sb)
            ut_sb = const.tile([P, HPC * D], F32, tag="ut_sb")
            nc.vector.tensor_copy(ut_sb, ut_ps)

            # ---- M_h = UT_h^T WvT_h, scaled on the scalar engine ----
            m_ps = ps_y.tile([P, HPC * D], F32, tag="c_ps")
            for h in range(HPC):
                sl = slice(h * D, (h + 1) * D)
                nc.tensor.matmul(m_ps[:, sl], lhsT=ut_sb[:, sl],
                                 rhs=wvt_sb[:, sl])
            m_all = const.tile([P, HPC * D], XT_DT, tag="m_all")
            nc.scalar.mul(m_all, m_ps, SCALE)

            # ---- finals + softmax epilogue ----
            for i in range(NCH):
                y_ps = ps_y.tile([P, HPC * D], F32, tag="c_ps")
                nc.tensor.matmul(y_ps, lhsT=xT_sb[:, i, :], rhs=m_all[:])

                negmax = small.tile([P, HPC], F32, tag="negmax")
                nc.vector.reduce_max(
                    out=negmax,
                    in_=y_ps[:].rearrange("p (h d) -> p h d", h=HPC),
                    axis=mybir.AxisListType.X, negate=True)

                t_sb = work.tile([P, HPC, D], F32, tag="t_sb")
                for h in range(HPC):
                    nc.scalar.activation(
                        t_sb[:, h, :], y_ps[:, h * D:(h + 1) * D],
                        mybir.ActivationFunctionType.Exp,
                        bias=negmax[:, h:h + 1], scale=1.0)

                sums = small.tile([P, HPC], F32, tag="sums")
                nc.vector.reduce_sum(out=sums, in_=t_sb,
                                     axis=mybir.AxisListType.X)
                rsum = small.tile([P, HPC], F32, tag="rsum")
                nc.vector.reciprocal(rsum, sums)

                o_sb = work.tile([P, HPC, D], BF16, tag="o_sb")
                nc.gpsimd.tensor_tensor(
                    o_sb, t_sb,
                    rsum[:, :, None].to_broadcast((P, HPC, D)),
                    mybir.AluOpType.mult)
                eng = nc.sync if i % 2 == 0 else nc.scalar
                eng.dma_start(out_d.ap()[i], o_sb)

    nc.compile()
    return nc


def _get_program():
    global _PROG
    if _PROG is None:
        _PROG = _build_program()
    return _PROG


def _make_in_maps(x, W_q, W_k, W_v):
    in_maps = []
    for core in range(N_CORES):
        b, hg = core // 2, core % 2
        sl = slice(hg * HPC * D, (hg + 1) * HPC * D)
        in_maps.append({
            "x": np.ascontiguousarray(x[b]),
            "wq": np.ascontiguousarray(W_q[sl]),
            "wk": np.ascontiguousarray(W_k[sl]),
            "wv": np.ascontiguousarray(W_v[sl]),
        })
    return in_maps


def run(x, W_q, W_k, W_v, trace=False, **spmd_kwargs):
    """Run on 8 NeuronCores; returns (Z, BassKernelResults)."""
    nc = _get_program()
    in_maps = _make_in_maps(np.asarray(x, np.float32), np.asarray(W_q, np.float32),
                            np.asarray(W_k, np.float32), np.asarray(W_v, np.float32))
    res = run_bass_kernel_spmd(nc, in_maps, core_ids=list(range(N_CORES)),
                               trace=trace, **spmd_kwargs)
    Z = np.empty((B, H, S, D), np.float32)
    for core in range(N_CORES):
        b, hg = core // 2, core % 2
        o = np.asarray(res.results[core]["out"]).astype(np.float32)
        # [NCH, P, HPC, D] -> [HPC, NCH*P, D]
        Z[b, hg * HPC:(hg + 1) * HPC] = o.transpose(2, 0, 1, 3).reshape(HPC, S, D)
    return Z, res


def kernel(x, W_q, W_k, W_v):
    Z, _ = run(x, W_q, W_k, W_v, trace=False)
    return Z


# revision 11
# speedup vs baseline: 1.5411x; 1.0350x over previous
"""Trainium2 Bass kernel for nn_MHA_2688649527670.

Reference computes, per batch b and head h:
    Q = x Wq_h^T, K = x Wk_h^T, V = x Wv_h^T          ([S, D] each)
    Z = softmax_over_d( (Q K^T / sqrt(D)) V )

No softmax between Q K^T and V, so the chain is associative:
    (Q K^T) V = x (Wq_h^T Wk_h G Wv_h^T) / sqrt(D),   G = x^T x   ([D, D])

which collapses the O(S^2 D) attention into a [D,D] weight chain plus one
[S,D]x[D,D*H] matmul, then softmax over d (free axis). Per-head softmax bias
is mandatory: per-head logit scales differ by >1000x, so a shared row max
underflows weak heads.

Sharding: batch (4) x head-groups (2x4 heads) = 8 independent cores.

Perf notes:
  - finals/UT in float32r: ~1 cycle/row at N=512 vs 4 for fp32, and HW
    measures ~1.5e-4 matmul error (~16x better than bf16; bf16 finals fail
    the 2e-2 gate at 3.1e-2, f32r lands ~2e-3). f32r operands must be
    WRITTEN as f32r by their producer (BIR rule); psum->sbuf copies do it.
  - PE p-state warmup matmuls on a memset tile from t~0 (streak -> 2.4GHz).
  - PE order: G (DMA-paced) with xT transposes interleaved, then P0T/WvT
    (weights arrive late on the gpsimd queue - off critical path), UT, M.
  - epilogue per chunk: V reduce_max -> 4x scalar Exp (per-head bias) ->
    V reduce_sum -> paired V reciprocal -> gpsimd normalize-mult (bf16) ->
    s-major contiguous bf16 DMA out (host reorders/upcasts).
"""

import numpy as np

import concourse.bass as bass
import concourse.bacc as bacc
import concourse.mybir as mybir
import concourse.tile as tile
from concourse.bass_utils import run_bass_kernel_spmd
from concourse.masks import make_identity

B, S, D, H = 4, 2048, 128, 8
P = 128
HPC = H // 2          # heads per core
NCH = S // P          # 16 s-chunks of 128 rows
N_CORES = 8
SCALE = 1.0 / float(np.sqrt(D))
F32 = mybir.dt.float32
F32R = mybir.dt.float32r
BF16 = mybir.dt.bfloat16

FINALS_F32R = True
UT_F32R = True
N_WARM = 4

_PROG = None


def _build_program():
    nc = bacc.Bacc("TRN2", target_bir_lowering=False, debug=False,
                   num_devices=N_CORES)

    x_d = nc.dram_tensor("x", [S, D], F32, kind="ExternalInput")
    wq_d = nc.dram_tensor("wq", [HPC * D, D], F32, kind="ExternalInput")
    wk_d = nc.dram_tensor("wk", [HPC * D, D], F32, kind="ExternalInput")
    wv_d = nc.dram_tensor("wv", [HPC * D, D], F32, kind="ExternalInput")
    # s-major output: [chunk, s-in-chunk, head, d], bf16; host reorders
    out_d = nc.dram_tensor("out", [NCH, P, HPC, D], BF16, kind="ExternalOutput")

    XT_DT = F32R if FINALS_F32R else F32

    with tile.TileContext(nc) as tc:
        with (
            tc.tile_pool(name="const", bufs=1) as const,
            tc.tile_pool(name="work", bufs=6) as work,
            tc.tile_pool(name="small", bufs=3) as small,
            tc.tile_pool(name="ps_y", bufs=4, space="PSUM") as ps_y,
            tc.tile_pool(name="ps_g", bufs=1, space="PSUM") as ps_g,
            tc.tile_pool(name="ps_t", bufs=3, space="PSUM") as ps_t,
        ):
            ident = const.tile([P, P], F32, tag="ident")
            make_identity(nc, ident)

            # ---- input DMAs: x on the two HW queues, weights on gpsimd ----
            x_sb = const.tile([P, NCH, D], F32, tag="x_sb")
            x_view = x_d.ap().rearrange("(n p) c -> p n c", p=P)
            for q in range(8):
                eng = nc.sync if q % 2 == 0 else nc.scalar
                eng.dma_start(x_sb[:, 2 * q:2 * q + 2, :],
                              x_view[:, 2 * q:2 * q + 2, :])
            w_sb = {}
            for nm, wd in (("wq", wq_d), ("wk", wk_d), ("wv", wv_d)):
                t = const.tile([P, HPC, D], F32, tag=f"{nm}_sb", name=f"{nm}_sb")
                nc.gpsimd.dma_start(t, wd.ap().rearrange("(h p) c -> p h c", p=P))
                w_sb[nm] = t

            # ---- G = x^T x (DMA-paced) with xT transposes interleaved.
            #      Transposes land 4-per-PSUM-bank; one V cast per bank
            #      writes the f32r xT tile (16 copies -> 4). ----
            g_ps = ps_g.tile([P, P], F32, tag="g_ps")
            xT_sb = const.tile([P, NCH, D], XT_DT, tag="xT_sb")
            tp_banks = {}

            def emit_xt(i):
                b = i // 4
                if i % 4 == 0:
                    tp_banks[b] = ps_t.tile([P, 4, P], F32, tag="tp", name=f"tp{b}")
                nc.tensor.transpose(tp_banks[b][:, i % 4, :],
                                    x_sb[:, i, :], ident)

            def emit_cast(b):
                eng = nc.vector if b % 2 == 0 else nc.scalar
                if b % 2 == 0:
                    nc.vector.tensor_copy(xT_sb[:, 4 * b:4 * b + 4, :],
                                          tp_banks[b])
                else:
                    nc.scalar.copy(xT_sb[:, 4 * b:4 * b + 4, :], tp_banks[b])

            p0t_ps = ps_y.tile([P, HPC * D], F32, tag="c_ps")
            for i in range(NCH):
                nc.tensor.matmul(g_ps, lhsT=x_sb[:, i, :], rhs=x_sb[:, i, :],
                                 start=(i == 0), stop=(i == NCH - 1))
                if i >= 4:
                    emit_xt(i - 4)
                if i >= 12:  # P0T woven into the G tail (weights ready)
                    h = i - 12
                    nc.tensor.matmul(p0t_ps[:, h * D:(h + 1) * D],
                                     lhsT=w_sb["wk"][:, h, :],
                                     rhs=w_sb["wq"][:, h, :])
            for i in range(NCH - 4, NCH):
                emit_xt(i)
            wvt_ps = ps_y.tile([P, HPC * D], F32, tag="c_ps")
            for h in range(HPC):
                nc.tensor.transpose(wvt_ps[:, h * D:(h + 1) * D],
                                    w_sb["wv"][:, h, :], ident)

            p0t_sb = const.tile([P, HPC * D], F32R if UT_F32R else F32,
                                tag="p0t_sb")
            nc.vector.tensor_copy(p0t_sb, p0t_ps)
            g_sb = const.tile([P, P], F32R if UT_F32R else F32, tag="g_sb")
            nc.vector.tensor_copy(g_sb, g_ps)
            wvt_sb = const.tile([P, HPC * D], F32, tag="wvt_sb")
            nc.vector.tensor_copy(wvt_sb, wvt_ps)

            # ---- UT = G @ P0T (G symmetric), one N=512 f32r matmul ----
            ut_ps = ps_y.tile([P, HPC * D], F32, tag="c_ps")
            nc.tensor.matmul(ut_ps, lhsT=g_sb, rhs=p0t_sb)
            ut_sb = const.tile([P, HPC * D], F32, tag="ut_sb")
            nc.vector.tensor_copy(ut_sb, ut_ps)

            # ---- M_h = UT_h^T WvT_h, scaled on the scalar engine ----
            m_ps = ps_y.tile([P, HPC * D], F32, tag="c_ps")
            for h in range(HPC):
                sl = slice(h * D, (h + 1) * D)
                nc.tensor.matmul(m_ps[:, sl], lhsT=ut_sb[:, sl],
                                 rhs=wvt_sb[:, sl])
            m_all = const.tile([P, HPC * D], XT_DT, tag="m_all")
            nc.scalar.mul(m_all, m_ps, SCALE)

            # xT psum->sbuf casts (V even banks, S odd) after chain copies
            for b in range(4):
                emit_cast(b)

            # ---- finals + software-pipelined softmax epilogue:
            #      V runs max_{i} before sum_{i-1} so it never idles on S ----
            t_live = {}

            def emit_front(i):
                y_ps = ps_y.tile([P, HPC * D], F32, tag="c_ps")
                nc.tensor.matmul(y_ps, lhsT=xT_sb[:, i, :], rhs=m_all[:])
                negmax = small.tile([P, HPC], F32, tag="negmax")
                nc.vector.reduce_max(
                    out=negmax,
                    in_=y_ps[:].rearrange("p (h d) -> p h d", h=HPC),
                    axis=mybir.AxisListType.X, negate=True)
                t_sb = work.tile([P, HPC, D], F32, tag="t_sb")
                for h in range(HPC):
                    nc.scalar.activation(
                        t_sb[:, h, :], y_ps[:, h * D:(h + 1) * D],
                        mybir.ActivationFunctionType.Exp,
                        bias=negmax[:, h:h + 1], scale=1.0)
                t_live[i] = t_sb

            def emit_back(i):
                t_sb = t_live.pop(i)
                sums = small.tile([P, HPC], F32, tag="sums")
                nc.vector.reduce_sum(out=sums, in_=t_sb,
                                     axis=mybir.AxisListType.X)
                rsum = small.tile([P, HPC], F32, tag="rsum")
                nc.vector.reciprocal(rsum, sums)
                o_sb = work.tile([P, HPC, D], BF16, tag="o_sb")
                nc.gpsimd.tensor_tensor(
                    o_sb, t_sb,
                    rsum[:, :, None].to_broadcast((P, HPC, D)),
                    mybir.AluOpType.mult)
                eng = nc.sync if i % 2 == 0 else nc.scalar
                eng.dma_start(out_d.ap()[i], o_sb)

            for i in range(NCH):
                emit_front(i)
                if i > 0:
                    emit_back(i - 1)
            emit_back(NCH - 1)

    nc.compile()
    return nc


def _get_program():
    global _PROG
    if _PROG is None:
        _PROG = _build_program()
    return _PROG


def _make_in_maps(x, W_q, W_k, W_v):
    in_maps = []
    for core in range(N_CORES):
        b, hg = core // 2, core % 2
        sl = slice(hg * HPC * D, (hg + 1) * HPC * D)
        in_maps.append({
            "x": np.ascontiguousarray(x[b]),
            "wq": np.ascontiguousarray(W_q[sl]),
            "wk": np.ascontiguousarray(W_k[sl]),
            "wv": np.ascontiguousarray(W_v[sl]),
        })
    return in_maps


def run(x, W_q, W_k, W_v, trace=False, **spmd_kwargs):
    """Run on 8 NeuronCores; returns (Z, BassKernelResults)."""
    nc = _get_program()
    in_maps = _make_in_maps(np.asarray(x, np.float32), np.asarray(W_q, np.float32),
                            np.asarray(W_k, np.float32), np.asarray(W_v, np.float32))
    res = run_bass_kernel_spmd(nc, in_maps, core_ids=list(range(N_CORES)),
                               trace=trace, **spmd_kwargs)
    Z = np.empty((B, H, S, D), np.float32)
    for core in range(N_CORES):
        b, hg = core // 2, core % 2
        o = np.asarray(res.results[core]["out"]).astype(np.float32)
        # [NCH, P, HPC, D] -> [HPC, NCH*P, D]
        Z[b, hg * HPC:(hg + 1) * HPC] = o.transpose(2, 0, 1, 3).reshape(HPC, S, D)
    return Z, res


def kernel(x, W_q, W_k, W_v):
    Z, _ = run(x, W_q, W_k, W_v, trace=False)
    return Z
